# revision 1
# baseline (speedup 1.0000x reference)
"""Trainium2 Bass kernel for nn_DAxialConv2d (deformable axial conv, B=8, C=64, 128x128).

Sharding: data-parallel, one batch sample per NeuronCore (8 cores).

Per-core pipeline (n = h*w = 16384 flat positions, wrapped layout: partition
p = n%128 = image col j, free col g = n//128 = image row i):
  1. x zero-padded to a 130x130 grid (bf16) in SBUF (host pre-pads).
  2. Offset/mask convs (1x3 / 3x1, 18 maps) = 5 PE matmul streams over shifted
     slabs, f32 PSUM accumulation, bias on ACT drain.
  3. Maps PE-transposed to wrapped [128, g] layout; index/weight math on DVE f32.
  4. PK table in DRAM, column-major grid: entry e(x', y') = x'*130 + y'
     (x' = x+1 in [0,128], y' = y+1 in [0,129]), token = horizontal pair
     [ch @ (y'-1, x'-1) | ch @ (y'-1, x')] (128 bf16 = 256B), zeros off-image.
     One dma_gather descriptor (elem 512B = entries idx, idx+1 = vertical pair)
     fetches all 4 bilinear corners: slots [TL | TR | BL | BR].
     idx = (xb+1)*130 + yb+1, xb/yb = clip(floor coords, -1, 127).
  5. Blend on DVE: val = sum_cor wt_cor * G_cor (bf16, weights in doubled-pair
     broadcast APs). Corner weights fold validity masks and the sigmoid mask.
  6. val [128 pos, 192 (k*64+c)] -> PE transpose -> [(k c), pos] bf16,
     einsum out[o, n] = wT.T @ valT + bias on PE/ACT.
  Pass 2 repeats 4-6 on x_h (offsets for both passes come from x).
"""
import sys

sys.path.insert(0, "/opt/trn_rl_repo")

import numpy as np
import ml_dtypes
from contextlib import ExitStack

import concourse.bass as bass
import concourse.bacc as bacc
import concourse.tile as tile
import concourse.mybir as mybir

F32 = mybir.dt.float32
BF16 = mybir.dt.bfloat16
I16 = mybir.dt.int16
Alu = mybir.AluOpType
Act = mybir.ActivationFunctionType

C = 64
H = W = 128
K = 3
HW = H * W            # 16384
GW = 130              # grid stride (both axes)
GRID = GW * GW        # padded image elems per channel
N_ENT = GW * GW       # PK entries (x' 0..129, y' 0..129; x'=129 col unused)
B_BLK = 2048          # tap points per gather block
N_BLK = HW // B_BLK   # 8
GG = B_BLK // 128     # 16 position groups per block
NC512 = 512

_CACHE = {}


def _bf(a):
    return np.ascontiguousarray(np.asarray(a).astype(ml_dtypes.bfloat16))


def _f32(a):
    return np.ascontiguousarray(np.asarray(a).astype(np.float32))


def _pack_weights(ins):
    d = {}
    wh = np.concatenate([_f32(ins["w_off_h"]).reshape(6, C, K),
                         _f32(ins["w_mask_h"]).reshape(3, C, K)], 0)
    wv = np.concatenate([_f32(ins["w_off_v"]).reshape(6, C, K),
                         _f32(ins["w_mask_v"]).reshape(3, C, K)], 0)
    d["wc_c"] = _bf(np.concatenate([wh[:, :, 1].T, wv[:, :, 1].T], 1))  # [C,18]
    d["wc_hlr"] = _bf(np.concatenate([wh[:, :, 0].T, wh[:, :, 2].T], 0))  # [2C,9]
    d["wc_vtb"] = _bf(np.concatenate([wv[:, :, 0].T, wv[:, :, 2].T], 0))  # [2C,9]
    d["bconv_h"] = _f32(np.concatenate([ins["b_off_h"], ins["b_mask_h"]]).reshape(9, 1))
    d["bconv_v"] = _f32(np.concatenate([ins["b_off_v"], ins["b_mask_v"]]).reshape(9, 1))
    w1 = _f32(ins["w_h"]).reshape(C, C, K)
    w2 = _f32(ins["w_v"]).reshape(C, C, K)
    d["w1T"] = _bf(w1.transpose(2, 1, 0).reshape(K * C, C))  # [(k c), o]
    d["w2T"] = _bf(w2.transpose(2, 1, 0).reshape(K * C, C))
    d["b1"] = _f32(ins["b_h"].reshape(C, 1))
    d["b2"] = _f32(ins["b_v"].reshape(C, 1))
    d["ident_bf"] = _bf(np.eye(128))
    d["ident_f32"] = _f32(np.eye(128))
    eye16 = np.zeros((16, 128), np.float32)
    for p in range(128):
        eye16[p % 16, p] = 1.0
    d["eye16"] = eye16
    d["rowc"] = _f32(np.broadcast_to(np.arange(128)[None, :], (128, 128)))
    d["colc"] = _f32(np.broadcast_to(np.arange(128)[:, None], (128, 128)))
    d["zeros"] = _bf(np.zeros(GW * 128, np.float32))
    return d


def _pad_x(x):
    xp = np.zeros((C, GW, GW), np.float32)
    xp[:, 1:H + 1, 1:W + 1] = x
    return _bf(xp.reshape(C, GRID))


INPUT_SPECS = {
    "xpad": ([C, GRID], BF16), "wc_c": ([C, 18], BF16),
    "wc_hlr": ([2 * C, 9], BF16), "wc_vtb": ([2 * C, 9], BF16),
    "bconv_h": ([9, 1], F32), "bconv_v": ([9, 1], F32),
    "w1T": ([K * C, C], BF16), "w2T": ([K * C, C], BF16),
    "b1": ([C, 1], F32), "b2": ([C, 1], F32),
    "ident_bf": ([128, 128], BF16), "ident_f32": ([128, 128], F32),
    "eye16": ([16, 128], F32), "rowc": ([128, 128], F32),
    "colc": ([128, 128], F32), "zeros": ([GW * 128], BF16),
}


def _ap(base, off, pattern):
    """Clone AP with explicit [step,count] dims (elements). For SBUF/PSUM the
    first pair is the partition dim whose step must equal the underlying row
    stride (bass convention) — substituted automatically from `base`."""
    a = base.copy()
    a.offset = base.offset + off
    pattern = [list(p) for p in pattern]
    if a.space in (bass.MemorySpace.SBUF, bass.MemorySpace.PSUM):
        pattern[0][0] = base.ap[0][0]
    a.ap.clear()
    for p in pattern:
        a.ap.append((int(p[0]), int(p[1])))
    return a


def build_program(dump=None):
    nc = bacc.Bacc("TRN2", target_bir_lowering=False)
    ins_d = {n: nc.dram_tensor(n, s, dt, kind="ExternalInput")
             for n, (s, dt) in INPUT_SPECS.items()}
    out_d = nc.dram_tensor("out", [C, HW], F32, kind="ExternalOutput")
    dumps = {}
    if dump == "maps":
        dumps["maps"] = nc.dram_tensor("d_maps", [18, HW], F32, kind="ExternalOutput")
    if dump == "idx":
        dumps["idx"] = nc.dram_tensor("d_idx", [128, 2 * K * 128], F32, kind="ExternalOutput")
        dumps["wtd"] = nc.dram_tensor("d_wtd", [128, 2 * K * 1024], F32, kind="ExternalOutput")
    if dump == "xh":
        dumps["xh"] = nc.dram_tensor("d_xh", [C, HW], F32, kind="ExternalOutput")

    with tile.TileContext(nc) as tc, ExitStack() as ctx:
        _emit(ctx, tc, ins_d, out_d, dumps)
    nc.compile()
    return nc


def _emit(ctx, tc, ins_d, out_d, dumps):
    nc = tc.nc
    v = nc.vector

    cpool = ctx.enter_context(tc.tile_pool(name="const", bufs=1))
    dram = ctx.enter_context(tc.tile_pool(name="scratch", bufs=1, space="DRAM"))

    sb = {}
    for name, (shape, dt) in INPUT_SPECS.items():
        if name in ("zeros", "xpad"):
            continue
        p, f = shape
        if p <= 128:
            t = cpool.tile([p, f], dt, tag=name, name=name)
            nc.sync.dma_start(t[:], ins_d[name][:])
            sb[name] = t
        else:
            hi = cpool.tile([128, f], dt, tag=name + "_hi", name=name + "_hi")
            lo = cpool.tile([p - 128, f], dt, tag=name + "_lo", name=name + "_lo")
            nc.sync.dma_start(hi[:], ins_d[name][0:128, :])
            nc.sync.dma_start(lo[:], ins_d[name][128:p, :])
            sb[name + "_hi"], sb[name + "_lo"] = hi, lo

    # x stacks: x2 = [xpad | xpad>>2] (h-conv tap pair), x2v = [xpad | xpad>>260]
    # (v-conv tap pair). Lower half doubles as the plain padded image.
    x2 = cpool.tile([128, GRID], BF16, tag="x2", name="x2")
    nc.sync.dma_start(x2[0:C, :], ins_d["xpad"][:])
    nc.sync.dma_start(x2[C:128, 0:GRID - 2], ins_d["xpad"][:, 2:GRID])
    xpad = x2  # [0:C] rows = padded image
    pk1 = dram.tile([N_ENT * 128], BF16, tag="pk1", name="pk1")
    pk2 = dram.tile([N_ENT * 128], BF16, tag="pk2", name="pk2")

    # ---- PK zero borders: rows y'=0 and y'=129 for x' in 0..128 ----------
    zsrc = ins_d["zeros"][:]
    for pk in (pk1, pk2):
        for yoff in (0, 129 * 128):
            nc.sync.dma_start(
                _ap(pk[:], yoff, [[GW * 128, 129], [1, 128]]),
                _ap(zsrc, 0, [[128, 129], [1, 128]]))

    # ---- offset/mask convs -> wrapped maps ------------------------------
    mapw = cpool.tile([128, 18 * 128], F32, tag="mapw", name="mapw")

    with ExitStack() as sctx:
        conv_ps = sctx.enter_context(tc.tile_pool(name="conv_ps", bufs=2, space="PSUM"))
        mt_ps = sctx.enter_context(tc.tile_pool(name="mt_ps", bufs=2, space="PSUM"))
        mchunk = sctx.enter_context(tc.tile_pool(name="mchunk", bufs=2))
        xvpool = sctx.enter_context(tc.tile_pool(name="xvpool", bufs=1))
        x2v = xvpool.tile([128, GRID], BF16, tag="x2v", name="x2v")
        nc.sync.dma_start(x2v[0:C, :], ins_d["xpad"][:])
        nc.sync.dma_start(x2v[C:128, 0:GRID - 260], ins_d["xpad"][:, 260:GRID])

        # chunks of 3 image rows, computed over the contiguous padded-grid
        # column space (outputs at pad columns are garbage and simply unused)
        for t in range((H + 2) // 3):
            s = (3 * t + 1) * GW
            e_end = min((3 * t + 4) * GW, H * GW + W + 1)  # needed e <= 16768
            N = e_end - s
            rows = [r for r in (3 * t, 3 * t + 1, 3 * t + 2) if r < H]
            ps_h = conv_ps.tile([9, 390], F32, tag="ps_h", name="ps_h")
            ps_v = conv_ps.tile([9, 390], F32, tag="ps_v", name="ps_v")

            mm = nc.tensor.matmul
            mm(ps_h[:, 0:N], sb["wc_hlr"][:], x2[:, s - 1:s - 1 + N],
               start=True, stop=False)
            mm(ps_h[:, 0:N], sb["wc_c"][:, 0:9], x2[0:C, s:s + N],
               start=False, stop=True)
            mm(ps_v[:, 0:N], sb["wc_vtb"][:], x2v[:, s - GW:s - GW + N],
               start=True, stop=False)
            mm(ps_v[:, 0:N], sb["wc_c"][:, 9:18], x2[0:C, s:s + N],
               start=False, stop=True)

            mc_h = mchunk.tile([9, 390], F32, tag="mc_h", name="mc_h")
            mc_v = mchunk.tile([9, 390], F32, tag="mc_v", name="mc_v")
            nc.scalar.activation(mc_h[:, 0:N], ps_h[:, 0:N], Act.Identity,
                                 bias=sb["bconv_h"][:])
            nc.scalar.activation(mc_v[:, 0:N], ps_v[:, 0:N], Act.Identity,
                                 bias=sb["bconv_v"][:])

            tp = mt_ps.tile([128, 3 * 18], F32, name="tp")
            for i, r in enumerate(rows):
                local = (r - 3 * t) * GW + 1
                if "maps" in dumps:
                    nc.sync.dma_start(dumps["maps"][0:9, r * W:(r + 1) * W],
                                      mc_h[:, local:local + W])
                    nc.sync.dma_start(dumps["maps"][9:18, r * W:(r + 1) * W],
                                      mc_v[:, local:local + W])
                nc.tensor.transpose(tp[:, i * 18:i * 18 + 9],
                                    mc_h[:, local:local + W],
                                    sb["ident_f32"][0:9, 0:9])
                nc.tensor.transpose(tp[:, i * 18 + 9:i * 18 + 18],
                                    mc_v[:, local:local + W],
                                    sb["ident_f32"][0:9, 0:9])
            # mapw is map-major: map m occupies cols [m*128, (m+1)*128), col g = row
            # tp free order (i, half, m9) -> mapw col (half*9+m9)*128 + 3t + i
            nc.scalar.activation(
                _ap(mapw[:], 3 * t, [[1, 128], [1, len(rows)], [9 * 128, 2], [128, 9]]),
                tp[:, 0:len(rows) * 18], Act.Identity)

    def mv(m):
        return mapw[:, m * 128:(m + 1) * 128]

    # ---- index/weight math ---------------------------------------------
    idxw = cpool.tile([128, 2 * K * 1024], I16, tag="idxw", name="idxw")
    wtd = cpool.tile([128, 2 * K * 1024], BF16, tag="wtd", name="wtd")
    taps = [-1.0, 0.0, 1.0]

    with ExitStack() as sctx:
        mpool = sctx.enter_context(tc.tile_pool(name="idxmath", bufs=2))
        f16 = sctx.enter_context(tc.tile_pool(name="fold16", bufs=2, space="PSUM"))
        f128 = sctx.enter_context(tc.tile_pool(name="fold128", bufs=2, space="PSUM"))

        def mt(tag):
            return mpool.tile([128, 128], F32, tag=tag, name="mt_" + tag)

        for pas in range(2):
            for k in range(K):
                if pas == 0:
                    dy_m, dx_m, msk_m = mv(2 * k), mv(2 * k + 1), mv(6 + k)
                    ky, kx = 0.0, taps[k]
                else:
                    dy_m, dx_m = mv(9 + 2 * k), mv(10 + 2 * k)
                    msk_m = mv(15 + k)
                    ky, kx = taps[k], 0.0

                py, px, wy, wx = mt("py"), mt("px"), mt("wy"), mt("wx")
                y0, x0 = mt("y0"), mt("x0")
                # py = i + ky + dy ; wrapped: i = rowc (free col), j = colc (partition)
                v.tensor_scalar(py[:], dy_m, ky, None, Alu.add)
                v.tensor_tensor(py[:], py[:], sb["rowc"][:], Alu.add)
                v.tensor_scalar(px[:], dx_m, kx, None, Alu.add)
                v.tensor_tensor(px[:], px[:], sb["colc"][:], Alu.add)
                # floor via round-half-even at 2^23: y0 = rne(py + 16 - 0.5 + 2^23)
                # - 2^23 - 16. Off-by-one at exact (half-)integers is harmless:
                # the bilinear endpoints coincide there (incl. mask boundaries).
                MAGIC = 8388608.0
                v.tensor_scalar(y0[:], py[:], 15.5, MAGIC, Alu.add, Alu.add)
                v.tensor_scalar(y0[:], y0[:], MAGIC, 16.0, Alu.subtract, Alu.subtract)
                v.tensor_tensor(wy[:], py[:], y0[:], Alu.subtract)
                v.tensor_scalar(x0[:], px[:], 15.5, MAGIC, Alu.add, Alu.add)
                v.tensor_scalar(x0[:], x0[:], MAGIC, 16.0, Alu.subtract, Alu.subtract)
                v.tensor_tensor(wx[:], px[:], x0[:], Alu.subtract)

                vy0, vy1, vx0, vx1 = mt("vy0"), mt("vy1"), mt("vx0"), mt("vx1")
                t0, t1 = mt("t0"), mt("t1")
                for vt, src, lo_thr, hi_thr in (
                        (vy0, y0, -0.5, 127.5), (vy1, y0, -1.5, 126.5),
                        (vx0, x0, -0.5, 127.5), (vx1, x0, -1.5, 126.5)):
                    v.tensor_scalar(t0[:], src[:], lo_thr, None, Alu.is_ge)
                    v.tensor_scalar(t1[:], src[:], hi_thr, None, Alu.is_le)
                    v.tensor_tensor(vt[:], t0[:], t1[:], Alu.mult)

                mk = mt("mk")
                nc.scalar.activation(mk[:], msk_m, Act.Sigmoid)

                ayt, ayb, axl, axr = mt("ayt"), mt("ayb"), mt("axl"), mt("axr")
                v.tensor_scalar(t0[:], wy[:], -1.0, 1.0, Alu.mult, Alu.add)
                v.tensor_tensor(ayt[:], t0[:], vy0[:], Alu.mult)
                v.tensor_tensor(ayb[:], wy[:], vy1[:], Alu.mult)
                v.tensor_scalar(t0[:], wx[:], -1.0, 1.0, Alu.mult, Alu.add)
                v.tensor_tensor(t0[:], t0[:], mk[:], Alu.mult)
                v.tensor_tensor(axl[:], t0[:], vx0[:], Alu.mult)
                v.tensor_tensor(t1[:], wx[:], mk[:], Alu.mult)
                v.tensor_tensor(axr[:], t1[:], vx1[:], Alu.mult)

                # slots [TL, TR, BL, BR]; doubled bf16 at [p, g*8 + cor*2 + rep]
                wbase = (pas * K + k) * 1024
                for cor, (ay, ax) in enumerate(
                        [(ayt, axl), (ayt, axr), (ayb, axl), (ayb, axr)]):
                    v.tensor_tensor(t0[:], ay[:], ax[:], Alu.mult)
                    for rep in range(2):
                        v.tensor_copy(
                            _ap(wtd[:], wbase + cor * 2 + rep, [[1, 128], [8, 128]]),
                            t0[:])

                yb, xb, idxf = mt("yb"), mt("xb"), mt("idxf")
                v.tensor_scalar(yb[:], y0[:], -1.0, 127.0, Alu.max, Alu.min)
                v.tensor_scalar(xb[:], x0[:], -1.0, 127.0, Alu.max, Alu.min)
                v.tensor_scalar(t0[:], xb[:], float(GW), None, Alu.mult)
                v.tensor_tensor(idxf[:], t0[:], yb[:], Alu.add)
                v.tensor_scalar(idxf[:], idxf[:], float(GW + 1), None, Alu.add)
                if "idx" in dumps:
                    nc.sync.dma_start(
                        dumps["idx"][:, (pas * K + k) * 128:(pas * K + k + 1) * 128],
                        idxf[:])

                # fold to 16-wrap via PE: W16[q, 8g+ph] = idxf[ph*16+q, g].
                # Fold matmuls write contiguous 64-col blocks (ph-major); a
                # small strided DVE copy reorders (ph, g) -> s = 8g+ph.
                w16s = mpool.tile([16, 1024], F32, tag="w16s", name="w16s")
                for half in range(2):
                    pf = f16.tile([16, 512], F32, name="pf")
                    for ph in range(8):
                        nc.tensor.matmul(
                            pf[:, ph * 64:(ph + 1) * 64],
                            sb["ident_f32"][:, ph * 16:(ph + 1) * 16],
                            idxf[:, half * 64:(half + 1) * 64],
                            start=(ph == 0), stop=(ph == 7), skip_group_check=True)
                    w16t = mpool.tile([16, 512], F32, tag="w16t", name="w16t")
                    nc.scalar.activation(w16t[:], pf[:], Act.Identity)
                    v.tensor_copy(
                        w16s[:, half * 512:(half + 1) * 512],
                        _ap(w16t[:], 0, [[512, 16], [1, 64], [64, 8]]))
                # replicate to 128 partitions + cast to i16
                for half in range(2):
                    pr = f128.tile([128, 512], F32, name="pr")
                    nc.tensor.matmul(pr[:], sb["eye16"][:],
                                     w16s[:, half * 512:(half + 1) * 512],
                                     start=True, stop=True)
                    v.tensor_copy(
                        idxw[:, wbase + half * 512:wbase + (half + 1) * 512], pr[:])

        if "wtd" in dumps:
            for j in range(2 * K):
                wtf = mpool.tile([128, 1024], F32, tag="wtf", name="wtf")
                v.tensor_copy(wtf[:], wtd[:, j * 1024:(j + 1) * 1024])
                nc.sync.dma_start(dumps["wtd"][:, j * 1024:(j + 1) * 1024], wtf[:])

    # ---- PK builders ----------------------------------------------------
    xt_pool = ctx.enter_context(tc.tile_pool(name="xt", bufs=2))
    ps_xt = ctx.enter_context(tc.tile_pool(name="ps_xt", bufs=2, space="PSUM"))

    def pk_build(pk, grid_sb):
        """grid_sb: [C, GRID] bf16 padded image (130x130, row y at grid row y+1).
        PK[x'*130+y'] = [ch @ (y'-1, x'-1) | ch @ (y'-1, x')]."""
        for rr in range(0, H, 32):  # 32 image rows per staging tile
            xt = xt_pool.tile([128, 32 * 128], BF16, tag="xt", name="xt")
            for r4 in range(0, 32, 4):
                psx = ps_xt.tile([128, 512], BF16, tag="psx", name="psx")
                for i in range(4):
                    y = rr + r4 + i
                    # T_a: cols x'-1 (grid cols 0..127); T_b: cols x' (1..128)
                    nc.tensor.transpose(
                        psx[:, i * 128:i * 128 + 64],
                        _ap(grid_sb[:], (y + 1) * GW + 0, [[1, C], [1, 128]]),
                        sb["ident_bf"][0:C, 0:C])
                    nc.tensor.transpose(
                        psx[:, i * 128 + 64:i * 128 + 128],
                        _ap(grid_sb[:], (y + 1) * GW + 1, [[1, C], [1, 128]]),
                        sb["ident_bf"][0:C, 0:C])
                nc.scalar.activation(xt[:, r4 * 128:(r4 + 4) * 128], psx[:],
                                     Act.Identity)
            # swap halves: psx holds [col x-1 | col x] per image-col partition p=x
            # fill entries e = p*130 + (y+1), y in [rr, rr+32)
            nc.sync.dma_start(
                _ap(pk[:], (rr + 1) * 128, [[GW * 128, 128], [128, 32], [1, 128]]),
                _ap(xt[:], 0, [[1, 128], [128, 32], [1, 128]]))
        # x' = 128 grid column: token = [ch @ (y, 127) | ch @ (y, 128)=0].
        # Transpose grid cols [128, 129] -> out [2, C]: partition 0 = col 128
        # (= img col 127), partition 1 = col 129 (zeros).
        for rr in range(0, H, 32):
            xe = xt_pool.tile([2, 32 * C], BF16, tag="xe", name="xe")
            for r4 in range(0, 32, 4):
                pse = ps_xt.tile([2, 4 * C], BF16, tag="pse", name="pse", bufs=1)
                for i in range(4):
                    y = rr + r4 + i
                    nc.tensor.transpose(
                        pse[:, i * C:(i + 1) * C],
                        _ap(grid_sb[:], (y + 1) * GW + 128, [[1, C], [1, 2]]),
                        sb["ident_bf"][0:C, 0:C])
                nc.scalar.activation(xe[:, r4 * C:(r4 + 4) * C], pse[:],
                                     Act.Identity)
            # entry e = 128*130 + (y+1): half = partition, addr = e*128 + half*64 + c
            nc.sync.dma_start(
                _ap(pk[:], 128 * GW * 128 + (rr + 1) * 128,
                    [[C, 2], [128, 32], [1, C]]),
                _ap(xe[:], 0, [[1, 2], [C, 32], [1, C]]))

    pk_build(pk1, xpad)

    # ---- main per-pass loop --------------------------------------------
    xhp = cpool.tile([C, GRID], BF16, tag="xhp", name="xhp")  # padded x_h grid
    v.memset(xhp[:], 0.0)  # borders must be zero; interior overwritten by pass 1

    gpool = ctx.enter_context(tc.tile_pool(name="gath", bufs=2))
    bpool = ctx.enter_context(tc.tile_pool(name="blend", bufs=2))
    vpool = ctx.enter_context(tc.tile_pool(name="val", bufs=2))
    vtpool = ctx.enter_context(tc.tile_pool(name="valT", bufs=2))
    opool = ctx.enter_context(tc.tile_pool(name="outs", bufs=3))
    ps_vt = ctx.enter_context(tc.tile_pool(name="ps_vt", bufs=2, space="PSUM"))
    ps_ein = ctx.enter_context(tc.tile_pool(name="ps_ein", bufs=2, space="PSUM"))

    for pas in range(2):
        pk = pk1 if pas == 0 else pk2
        wT_hi = sb["w1T_hi"] if pas == 0 else sb["w2T_hi"]
        wT_lo = sb["w1T_lo"] if pas == 0 else sb["w2T_lo"]
        bias = sb["b1"] if pas == 0 else sb["b2"]
        # max idx = 128*130+128 = 16768; covering reads entry 16769. Declaring
        # 16770 entries keeps the AP span (16769*128+256) inside the buffer.
        pk_src = _ap(pk[:], 0, [[128, 128 * GW + 129], [1, 256]])

        for blk in range(N_BLK):
            val = vpool.tile([128, GG * K * C], BF16, tag="val", name="val")
            for k in range(K):
                g_t = gpool.tile([128, GG * 256], BF16, tag="g", name="g_t")
                tb_ = (pas * K + k) * 1024
                idxs = idxw[:, tb_ + blk * 128:tb_ + (blk + 1) * 128]
                nc.gpsimd.dma_gather(
                    _ap(g_t[:], 0, [[1, 128], [256, GG], [1, 256]]),
                    pk_src, idxs, B_BLK, B_BLK, 256, elem_step=128, single_packet=False)

                ta = bpool.tile([128, GG * C], BF16, tag="ta", name="ta")
                tb = bpool.tile([128, GG * C], BF16, tag="tb", name="tb")
                tcc = bpool.tile([128, GG * C], BF16, tag="tc", name="tcc")

                def wt_ap(cor, tb_=tb_):
                    return _ap(wtd[:], tb_ + blk * GG * 8 + cor * 2,
                               [[1, 128], [8, GG], [0, 32], [1, 2]])

                def g_ap(cor):
                    return _ap(g_t[:], cor * 64, [[1, 128], [256, GG], [2, 32], [1, 2]])

                def t4(t):
                    return _ap(t[:], 0, [[1, 128], [C, GG], [2, 32], [1, 2]])

                v.tensor_tensor(t4(ta), g_ap(0), wt_ap(0), Alu.mult)
                v.tensor_tensor(t4(tb), g_ap(1), wt_ap(1), Alu.mult)
                v.tensor_tensor(ta[:], ta[:], tb[:], Alu.add)
                v.tensor_tensor(t4(tb), g_ap(2), wt_ap(2), Alu.mult)
                v.tensor_tensor(t4(tcc), g_ap(3), wt_ap(3), Alu.mult)
                v.tensor_tensor(tb[:], tb[:], tcc[:], Alu.add)
                v.tensor_tensor(
                    _ap(val[:], k * C, [[1, 128], [K * C, GG], [1, C]]),
                    _ap(ta[:], 0, [[1, 128], [C, GG], [1, C]]),
                    _ap(tb[:], 0, [[1, 128], [C, GG], [1, C]]), Alu.add)

            vT_hi = vtpool.tile([128, B_BLK], BF16, tag="vth", name="vT_hi")
            vT_lo = vtpool.tile([C, B_BLK], BF16, tag="vtl", name="vT_lo")
            vv = val[:].rearrange("p (g f) -> p g f", f=K * C)
            for g4 in range(GG // 4):
                # one 2KB psum bank: [:, 0:512] = hi transposes, [0:64, 512:1024] = lo
                php = ps_vt.tile([128, 1024], BF16, tag="php", name="php")
                for i in range(4):
                    g = g4 * 4 + i
                    nc.tensor.transpose(php[:, i * 128:(i + 1) * 128],
                                        vv[:, g, 0:128], sb["ident_bf"][:])
                    nc.tensor.transpose(php[0:C, 512 + i * 128:512 + (i + 1) * 128],
                                        vv[:, g, 128:192],
                                        sb["ident_bf"][:])
                nc.scalar.activation(vT_hi[:, g4 * 512:(g4 + 1) * 512],
                                     php[:, 0:512], Act.Identity)
                nc.scalar.activation(vT_lo[:, g4 * 512:(g4 + 1) * 512],
                                     php[0:C, 512:1024], Act.Identity)

            for s in range(B_BLK // NC512):
                pse = ps_ein.tile([C, NC512], F32, name="pse")
                nc.tensor.matmul(pse[:], wT_hi[:],
                                 vT_hi[:, s * NC512:(s + 1) * NC512],
                                 start=True, stop=False)
                nc.tensor.matmul(pse[:], wT_lo[:],
                                 vT_lo[:, s * NC512:(s + 1) * NC512],
                                 start=False, stop=True)
                n0 = blk * B_BLK + s * NC512
                if pas == 1:
                    ot = opool.tile([C, NC512], F32, tag="ot", name="ot")
                    nc.scalar.activation(ot[:], pse[:], Act.Identity, bias=bias[:])
                    nc.sync.dma_start(out_d[:, n0:n0 + NC512], ot[:])
                else:
                    # drain into padded x_h grid rows (4 rows of 128)
                    r0 = n0 // 128
                    nc.scalar.activation(
                        _ap(xhp[:], (r0 + 1) * GW + 1, [[1, C], [GW, 4], [1, W]]),
                        pse[:], Act.Identity, bias=bias[:])
                    if "xh" in dumps:
                        xhf = opool.tile([C, NC512], F32, tag="xhf", name="xhf")
                        nc.scalar.activation(xhf[:], pse[:], Act.Identity,
                                             bias=bias[:])
                        nc.sync.dma_start(dumps["xh"][:, n0:n0 + NC512], xhf[:])

        if pas == 0:
            pk_build(pk2, xhp)


def kernel(**inputs):
    from concourse.bass_utils import run_bass_kernel_spmd

    if "main" not in _CACHE:
        _CACHE["main"] = build_program()
    nc = _CACHE["main"]

    packed = _pack_weights(inputs)
    x = np.asarray(inputs["x"], np.float32)
    in_maps = []
    for core in range(8):
        m = dict(packed)
        m["xpad"] = _pad_x(x[core])
        in_maps.append(m)

    res = run_bass_kernel_spmd(nc, in_maps, core_ids=list(range(8)))
    outs = [np.asarray(res.results[i]["out"]).reshape(C, H, W) for i in range(8)]
    return np.stack(outs).astype(np.float32)

